# revision 33
# baseline (speedup 1.0000x reference)
"""Trainium2 Bass kernel for nn_EquivariantDiffuserV46 (GNN message passing).

Computation (node-MLP branch of the reference is dead code — output depends
only on the coord path):
    h = concat(cond, t)                    [BN, 64]
    edge_attr = silu(d*ew1+eb1) @ ew2+eb2  [E, 32]
    m = [h[src], h[dst], edge_attr]        [E, 160]
    cw = silu(m @ cw1 + cb1) @ cw2         [E, 1]
    upd = cw * (x[src]-x[dst]) / max(||x[src]-x[dst]||, 1e-8)
    out = x + segment_sum(upd, dst)

Strategy (v3): edges sorted by dst, dst-range sharded over 8 cores. Each
core fetches per-src rows [A=h@cw1[:64] bf16(128) | x f32(3)] (512B) with
batched dma_gather instructions striped over 4 SWDGE queues — one gather
per 2048-edge tile. Gathering the projection A instead of h lets the
src contribution enter the z PSUM via a single lhsT=A/rhs=identity
matmul (no transposes, no PSUM->SBUF copies). The dst side needs no
gather: edges are laid out with a compile-time-fixed number of chunks
per 64-node dst block, so h[dst]-dependent terms come from per-block
tables (Btab = Hblk @ cw1[64:128] f32, xfbT f32) via one-hot matmuls.
1/len uses the DVE bit-trick rsqrt (+2 Newton steps) so the ACT engine
only ever runs Silu (one table set, no table thrash). dma_gather
indices are int16, so chunks are split into two src classes (gather
base 0 vs BN-32768); class regions are tile-aligned.
"""
import os
import sys

for _p in ("/opt/trn_rl_repo",):
    if _p not in sys.path:
        sys.path.insert(0, _p)

import numpy as np
import ml_dtypes

from concourse import bass, mybir
from concourse.tile import TileContext
from concourse.masks import make_identity
from concourse import library_config
from concourse.library_overlay import lower_extended_insts

F32 = mybir.dt.float32
BF16 = mybir.dt.bfloat16
I32 = mybir.dt.int32
I16 = mybir.dt.int16
P = 128          # partitions / edges per chunk
BLK = 64         # nodes per dst block
CHT = 16         # chunks per tile (2048 edges per gather)
ROWE = 256       # gather row elements (bf16): 512B
N_CORES = 8
NQ = 4           # SWDGE queues for gathers
RSQRT_MAGIC = 0x5F3759DF


# ---------------------------------------------------------------- host prep

def _plan(src, dst, edge_dist, BN, n_cores):
    """Sort edges by dst, shard by dst range, lay out a uniform chunk stream.

    Chunk stream layout (identical shape for all cores; data differs):
      [ nblk blocks x CPL low-class chunks | tile pad | nblk x CPH high | pad ]
    Chunk c < TLC belongs to block c // CPL; chunk c >= TLC to block
    (c - TLC) // CPH (clamped). Every chunk holds 128 edges of one dst
    block and one src class; pad edges have dstloc=99 (dead one-hot).
    """
    n_core = BN // n_cores
    nblk = (n_core + BLK - 1) // BLK
    hi_base = BN - 32768            # high-class gather base
    lo_cut = hi_base                # src < lo_cut  -> rigid low
    hi_cut = 32768                  # src >= hi_cut -> rigid high

    order = np.argsort(dst, kind="stable")
    src_s = src[order]
    dst_s = dst[order]
    dist_s = edge_dist[order]
    bounds = np.searchsorted(dst_s, np.arange(0, BN + 1, n_core))

    percore = []
    need_l = need_h = need_t = 0
    for c in range(n_cores):
        lo, hi = bounds[c], bounds[c + 1]
        base = c * n_core
        cs, cd, cw = src_s[lo:hi], dst_s[lo:hi], dist_s[lo:hi]
        blk = (cd - base) // BLK
        rows = []
        for b in range(nblk):
            m = blk == b
            bs, bd, bw = cs[m], cd[m], cw[m]
            cls_rl = bs < lo_cut
            cls_rh = bs >= hi_cut
            cls_fx = ~cls_rl & ~cls_rh
            rows.append((bs, bd, bw, cls_rl, cls_rh, cls_fx))
            nl, nh, nf = int(cls_rl.sum()), int(cls_rh.sum()), int(cls_fx.sum())
            need_l = max(need_l, nl)
            need_h = max(need_h, nh)
            need_t = max(need_t, nl + nh + nf)
        percore.append((base, rows))

    CPL = (need_l + P - 1) // P
    CPH = (need_h + P - 1) // P
    while CPL * P + CPH * P < need_t:
        if CPL <= CPH:
            CPL += 1
        else:
            CPH += 1

    ncl = nblk * CPL
    nch = nblk * CPH
    TL = (ncl + CHT - 1) // CHT
    TH = (nch + CHT - 1) // CHT
    NT = TL + TH
    if NT % 8:                            # nslot = nchunk/128 integral
        TH += 8 - NT % 8
        NT = TL + TH
    nchunk = NT * CHT
    TLC = TL * CHT

    def blk_of_chunk(c):
        if c < TLC:
            return min(c // CPL, nblk - 1)
        return min((c - TLC) // CPH, nblk - 1)

    metas = []
    for base, rows in percore:
        idx = np.zeros(nchunk * P, np.int64)
        dloc = np.full(nchunk * P, 99.0, np.float32)
        dw = np.ones(nchunk * P, np.float32)
        for b in range(nblk):
            bs, bd, bw, rl, rh, fx = rows[b]
            nl_r = int(rl.sum())
            x_lo = min(int(fx.sum()), CPL * P - nl_r)
            fx_idx = np.nonzero(fx)[0]
            lo_m = rl.copy()
            lo_m[fx_idx[:x_lo]] = True
            hi_m = ~lo_m
            for cls, m, cp, coff, cbase in (
                (0, lo_m, CPL, b * CPL * P, 0),
                (1, hi_m, CPH, TLC * P + b * CPH * P, hi_base),
            ):
                es, ed, ew = bs[m], bd[m], bw[m]
                n = es.size
                assert n <= cp * P, (b, cls, n, cp * P)
                sl = slice(coff, coff + n)
                idx[sl] = es - cbase
                dloc[sl] = (ed - (base + b * BLK)).astype(np.float32)
                dw[sl] = ew

        idx16 = idx.astype(np.int16)
        assert (idx < 32768).all() and (idx >= 0).all()
        iw = np.zeros((P, NT * P), np.int16)
        for t in range(NT):
            blk16 = idx16[t * CHT * P:(t + 1) * CHT * P].reshape(CHT * P // 16, 16).T
            for rep in range(8):
                iw[rep * 16:(rep + 1) * 16, t * P:(t + 1) * P] = blk16

        bid = np.array([blk_of_chunk(c) for c in range(nchunk)], np.float32)
        metas.append(dict(
            idx16=np.ascontiguousarray(iw),
            dl=np.ascontiguousarray(dloc.reshape(nchunk, P).T),
            dlrow=np.ascontiguousarray(dloc.reshape(1, -1)),
            drow=np.ascontiguousarray(dw.reshape(1, -1).astype(ml_dtypes.bfloat16)),
            dlrow_bf=np.ascontiguousarray(
                dloc.reshape(1, -1).astype(ml_dtypes.bfloat16)),
            blockid=np.ascontiguousarray(bid.reshape(nchunk // P, P).T),
            base=base,
        ))
    return metas, nchunk, nblk, n_core, TL, hi_base, CPL, CPH


# ------------------------------------------------------------- bass builder

def _split_ctrl_waits(nc, limit=1):
    """Walrus in this toolchain rejects >limit sync waits on Drain-style ctrl
    instructions; move overflow waits onto preceding same-engine NoOps."""
    import bass_rust
    for fn in nc.m.functions:
        for bb in fn.blocks:
            out = []
            for inst in bb.instructions:
                si = inst.sync_info
                if (si is not None and si.on_wait
                        and len(si.on_wait) > limit):
                    waits = list(si.on_wait)
                    ups = list(si.on_update) if si.on_update else []
                    head, tail = waits[:-limit], waits[-limit:]
                    for k in range(0, len(head), limit):
                        nop = mybir.InstNoOp(name=f"{inst.name}-w{k}", ins=[], outs=[])
                        nop.engine = inst.engine
                        nop.sync_info = bass_rust.SyncInfo(
                            on_wait=head[k:k + limit], on_update=[])
                        out.append(nop)
                    inst.sync_info = bass_rust.SyncInfo(on_wait=tail, on_update=ups)
                out.append(inst)
            bb.instructions = out


def build_bass(BN, nchunk, nblk, TL, hi_base, CPL, CPH, n_cores=N_CORES):
    NT = nchunk // CHT
    nslot = nchunk // P
    epad = nchunk * P
    TLC = TL * CHT

    nc = bass.Bass("TRN2", target_bir_lowering=False, debug=False,
                   num_devices=n_cores, num_swdge_queues=NQ)

    table = nc.dram_tensor("table", [BN, ROWE], BF16, kind="ExternalInput")
    idx16 = nc.dram_tensor("idx16", [P, NT * P], I16, kind="ExternalInput")
    dl = nc.dram_tensor("dl", [P, nchunk], F32, kind="ExternalInput")
    dlrow = nc.dram_tensor("dlrow", [1, epad], BF16, kind="ExternalInput")
    drow = nc.dram_tensor("drow", [1, epad], BF16, kind="ExternalInput")
    blockid = nc.dram_tensor("blockid", [P, nslot], F32, kind="ExternalInput")
    xfb = nc.dram_tensor("xfb", [nblk, 192], F32, kind="ExternalInput")
    xfbt = nc.dram_tensor("xfbt", [BLK, nblk * 9], BF16, kind="ExternalInput")
    btab = nc.dram_tensor("btab", [BLK, nblk * 128], BF16, kind="ExternalInput")
    w2c = nc.dram_tensor("w2c", [32, 128], BF16, kind="ExternalInput")
    ew1c = nc.dram_tensor("ew1c", [1, 32], BF16, kind="ExternalInput")
    cw2c = nc.dram_tensor("cw2c", [128, 1], BF16, kind="ExternalInput")
    eb1c = nc.dram_tensor("eb1c", [32, 1], F32, kind="ExternalInput")
    cb1p = nc.dram_tensor("cb1p", [128, 1], F32, kind="ExternalInput")
    yout = nc.dram_tensor("yout", [nblk, 192], F32, kind="ExternalOutput")

    AF = mybir.ActivationFunctionType
    OP = mybir.AluOpType

    with TileContext(nc) as tc:
        with (
            tc.tile_pool(name="cst", bufs=1) as cst,
            tc.tile_pool(name="sb", bufs=2) as sbp,
            tc.tile_pool(name="gs", bufs=6) as gsp,
            tc.tile_pool(name="ps2", bufs=2, space="PSUM") as psp,
            tc.tile_pool(name="psxa", bufs=1, space="PSUM") as psxa,
            tc.tile_pool(name="ps1", bufs=1, space="PSUM") as psp1,
            tc.tile_pool(name="dr", bufs=1, space="DRAM") as drp,
        ):
            # ---------------- phase A: constants
            # (all gpsimd standard-library ops must precede load_library(mlp):
            # the reload replaces Q7 IRAM, and iota ucode lives in the
            # standard library)
            ident_bf = cst.tile([P, P], BF16)
            make_identity(nc, ident_bf)
            idx_sb = cst.tile([P, NT * P], I16)
            nc.sync.dma_start(out=idx_sb[:], in_=idx16[:])
            dl_sb = cst.tile([P, nchunk, 1], F32)
            nc.sync.dma_start(out=dl_sb[:, :, 0], in_=dl[:])
            blockid_sb = cst.tile([P, nslot], F32)
            nc.sync.dma_start(out=blockid_sb[:], in_=blockid[:])
            xfb_sb = cst.tile([nblk, 192], F32)
            nc.sync.dma_start(out=xfb_sb[:], in_=xfb[:])
            xfbt_sb = cst.tile([BLK, nblk * 9], BF16)
            nc.sync.dma_start(out=xfbt_sb[:], in_=xfbt[:])
            btab_sb = cst.tile([BLK, nblk * 128], BF16)
            nc.sync.dma_start(out=btab_sb[:], in_=btab[:])
            w2c_sb = cst.tile([32, 128], BF16)
            nc.sync.dma_start(out=w2c_sb[:], in_=w2c[:])
            ew1c_sb = cst.tile([1, 32], BF16)
            nc.sync.dma_start(out=ew1c_sb[:], in_=ew1c[:])
            cw2c_sb = cst.tile([128, 1], BF16)
            nc.sync.dma_start(out=cw2c_sb[:], in_=cw2c[:])
            eb1c_sb = cst.tile([32, 1], F32)
            nc.sync.dma_start(out=eb1c_sb[:], in_=eb1c[:])
            cb1p_sb = cst.tile([128, 1], F32)
            nc.sync.dma_start(out=cb1p_sb[:], in_=cb1p[:])

            iota64i = cst.tile([P, 1, BLK], I32)
            nc.gpsimd.iota(iota64i[:, 0, :], pattern=[[1, BLK]], base=0,
                           channel_multiplier=0)
            iota64 = cst.tile([P, 1, BLK], F32)
            nc.vector.tensor_copy(iota64[:], iota64i[:])
            iotaci = cst.tile([BLK, 1, 1], I32)
            nc.gpsimd.iota(iotaci[:, 0, :], pattern=[[1, 1]], base=0,
                           channel_multiplier=1)
            iotac = cst.tile([BLK, 1, 1], BF16)
            nc.vector.tensor_copy(iotac[:], iotaci[:])
            iotabi = cst.tile([P, nblk], I32)
            nc.gpsimd.iota(iotabi[:], pattern=[[1, nblk]], base=0,
                           channel_multiplier=0)
            iotab = cst.tile([P, nblk], F32)
            nc.vector.tensor_copy(iotab[:], iotabi[:])
            ones1 = cst.tile([1, BLK], BF16)
            nc.vector.memset(ones1[:], 1.0)
            magic = cst.tile([P, 1], I32)
            nc.vector.memset(magic[:], RSQRT_MAGIC)
            nidx_reg = nc.gpsimd.to_reg(CHT * P)   # shared by all gathers
            nc.gpsimd.load_library(library_config.mlp)

            ydram = drp.tile([nchunk, 192], F32)

            # ---------------- phase B: edge tiles
            NT_dbg = int(os.environ.get("GNN_MAXT", str(NT)))
            for t in range(min(NT, NT_dbg)):
                c0 = t * CHT
                in_ap = table[:] if t < TL else table[hi_base:, :]
                Gs = gsp.tile([P, CHT, ROWE], BF16, tag="Gs")
                nc.gpsimd.dma_gather(
                    out_ap=Gs[:], in_ap=in_ap,
                    idxs_ap=idx_sb[:, t * P:(t + 1) * P],
                    num_idxs=CHT * P, num_idxs_reg=nidx_reg, elem_size=ROWE,
                    transpose=False, single_packet=False, queue_num=t % NQ)
                dr_t = sbp.tile([1, CHT * P], BF16, tag="dr_t")
                nc.sync.dma_start(out=dr_t[:], in_=drow[:, c0 * P:(c0 + CHT) * P])
                dlr_t = sbp.tile([1, CHT * P], BF16, tag="dlr_t")
                nc.sync.dma_start(out=dlr_t[:], in_=dlrow[:, c0 * P:(c0 + CHT) * P])

                cwxd_ps = psp.tile([P, CHT, 12], F32, tag="cwxd")
                st_all = sbp.tile([BLK, CHT, P], BF16, tag="st")
                s_all = sbp.tile([P, CHT, BLK], BF16, tag="sall")
                for h2 in range(2):
                    dlrep_ps = psp1.tile([BLK, CHT * P // 2], F32, tag="dlrep")
                    for g2 in range(2):
                        o = (h2 * 2 + g2) * 512
                        nc.tensor.matmul(out=dlrep_ps[:, g2 * 512:(g2 + 1) * 512],
                                         lhsT=ones1[:],
                                         rhs=dlr_t[0:1, o:o + 512],
                                         start=True, stop=True)
                    nc.vector.tensor_tensor(
                        out=st_all[:, h2 * 8:(h2 + 1) * 8, :],
                        in0=dlrep_ps[:].rearrange("p (a b) -> p a b", a=CHT // 2),
                        in1=iotac[:].to_broadcast([BLK, CHT // 2, P]),
                        op=OP.is_equal)
                nc.vector.tensor_tensor(
                    out=s_all[:],
                    in0=iota64[:].to_broadcast([P, CHT, BLK]),
                    in1=dl_sb[:, c0:c0 + CHT, :].to_broadcast([P, CHT, BLK]),
                    op=OP.is_equal)

                for g in range(4):
                    u_ps = psp1.tile([32, 512], F32, tag="u")
                    nc.tensor.matmul(out=u_ps[:], lhsT=ew1c_sb[:],
                                     rhs=dr_t[0:1, g * 512:(g + 1) * 512],
                                     start=True, stop=True)
                    u_sb = sbp.tile([32, 512], BF16, tag="u_sb")
                    nc.scalar.activation(u_sb[:], u_ps[:], AF.Silu, bias=eb1c_sb[:])

                    z_ps = psp.tile([P, 512], F32, tag="z")
                    for c4 in range(4):
                        cc = g * 4 + c4
                        c = c0 + cc
                        if c < TLC:
                            b = min(c // CPL, nblk - 1)
                        else:
                            b = min((c - TLC) // CPH, nblk - 1)
                        # src contribution: z[:, e] += A[src_e]^T via identity
                        nc.tensor.matmul(
                            out=z_ps[:, c4 * P:(c4 + 1) * P],
                            lhsT=Gs[:, cc, 0:128], rhs=ident_bf[:],
                            start=(c4 == 0), stop=False)
                        st = st_all[:, cc, :]
                        nc.tensor.matmul(out=z_ps[:, c4 * P:(c4 + 1) * P],
                                         lhsT=btab_sb[:, b * 128:(b + 1) * 128],
                                         rhs=st, start=False, stop=False)
                        nc.tensor.matmul(out=cwxd_ps[:, cc, 0:9],
                                         lhsT=st,
                                         rhs=xfbt_sb[:, b * 9:(b + 1) * 9],
                                         start=True, stop=True)
                    nc.tensor.matmul(out=z_ps[:], lhsT=w2c_sb[:], rhs=u_sb[:],
                                     start=False, stop=True)
                    w_sb = sbp.tile([P, 512], BF16, tag="w_sb")
                    nc.scalar.activation(w_sb[:], z_ps[:], AF.Silu, bias=cb1p_sb[:])
                    for c4 in range(4):
                        cc = g * 4 + c4
                        nc.tensor.matmul(out=cwxd_ps[:, cc, 9:10],
                                         lhsT=w_sb[:, c4 * P:(c4 + 1) * P],
                                         rhs=cw2c_sb[:], start=True, stop=True)

                # coord update (row layout, all DVE — no ACT tables)
                dirt = sbp.tile([P, CHT, 3], F32, tag="dirt")
                nc.vector.tensor_tensor(
                    out=dirt[:], in0=Gs[:, :, 128:134].bitcast(F32),
                    in1=cwxd_ps[:, :, 0:3], op=OP.subtract)
                nc.vector.tensor_tensor(
                    out=dirt[:], in0=dirt[:], in1=cwxd_ps[:, :, 3:6],
                    op=OP.subtract)
                nc.vector.tensor_tensor(
                    out=dirt[:], in0=dirt[:], in1=cwxd_ps[:, :, 6:9],
                    op=OP.subtract)
                sq = sbp.tile([P, CHT, 3], F32, tag="sq")
                nc.vector.tensor_tensor(out=sq[:], in0=dirt[:], in1=dirt[:],
                                        op=OP.mult)
                ss = sbp.tile([P, CHT, 1], F32, tag="ss")
                nc.vector.tensor_reduce(out=ss[:, :, 0], in_=sq[:],
                                        axis=mybir.AxisListType.X, op=OP.add)
                # 1/sqrt(ss): clamp+shift fused in int domain (positive f32
                # ordering == int ordering), bit-trick seed + 2 Newton iters
                yv = sbp.tile([P, CHT, 1], F32, tag="yv")
                nc.vector.tensor_scalar_max(ss[:, :, 0], ss[:, :, 0], 1e-16)
                nc.vector.tensor_scalar(
                    out=yv[:, :, 0].bitcast(I32), in0=ss[:, :, 0].bitcast(I32),
                    scalar1=1, scalar2=None, op0=OP.logical_shift_right)
                nc.vector.tensor_tensor(
                    out=yv[:, :, 0].bitcast(I32),
                    in0=magic[:].to_broadcast([P, CHT]),
                    in1=yv[:, :, 0].bitcast(I32), op=OP.subtract)
                tv = sbp.tile([P, CHT, 1], F32, tag="tv")
                for _ in range(2):
                    nc.vector.tensor_tensor(out=tv[:], in0=yv[:], in1=yv[:],
                                            op=OP.mult)
                    nc.vector.tensor_tensor(out=tv[:], in0=tv[:], in1=ss[:],
                                            op=OP.mult)
                    nc.vector.tensor_scalar(
                        out=tv[:, :, 0], in0=tv[:, :, 0], scalar1=-0.5,
                        scalar2=1.5, op0=OP.mult, op1=OP.add)
                    nc.vector.tensor_tensor(out=yv[:], in0=yv[:], in1=tv[:],
                                            op=OP.mult)
                fac = sbp.tile([P, CHT, 1], F32, tag="fac")
                nc.vector.tensor_tensor(out=fac[:, :, 0], in0=yv[:, :, 0],
                                        in1=cwxd_ps[:, :, 9], op=OP.mult)
                upd_bf = sbp.tile([P, CHT, 3], BF16, tag="upd_bf")
                nc.vector.tensor_tensor(out=upd_bf[:], in0=dirt[:],
                                        in1=fac[:].to_broadcast([P, CHT, 3]),
                                        op=OP.mult)

                # one-hot scatter -> per-chunk [3, 64] node sums
                ystrip = sbp.tile([3, CHT, BLK], F32, tag="ystrip")
                for h in range(2):
                    xa_ps = psxa.tile([3, 8 * BLK], F32, tag="xa")
                    for c8 in range(8):
                        cc = h * 8 + c8
                        nc.tensor.matmul(out=xa_ps[:, c8 * BLK:(c8 + 1) * BLK],
                                         lhsT=upd_bf[:, cc, :],
                                         rhs=s_all[:, cc, :],
                                         start=True, stop=True)
                    nc.scalar.copy(ystrip[:, h * 8:(h + 1) * 8, :], xa_ps[:])
                nc.sync.dma_start(
                    out=ydram[c0:c0 + CHT, :].rearrange("q (k j) -> k q j", k=3),
                    in_=ystrip[:])

            # ---------------- phase C: block-stage reduction + x residual
            ysb = cst.tile([P, nslot, 192], F32)
            nc.sync.dma_start(out=ysb[:],
                              in_=ydram[:].rearrange("(s p) f -> p s f", p=P))
            out_ps = psp.tile([nblk, 192], F32, tag="z")
            for s in range(nslot):
                O = sbp.tile([P, nblk], F32, tag="O")
                nc.vector.tensor_tensor(
                    out=O[:], in0=iotab[:],
                    in1=blockid_sb[:, s:s + 1].to_broadcast([P, nblk]),
                    op=OP.is_equal)
                nc.tensor.matmul(out=out_ps[:], lhsT=O[:], rhs=ysb[:, s, :],
                                 start=(s == 0), stop=(s == nslot - 1))
            yfin = cst.tile([nblk, 192], F32)
            nc.vector.tensor_tensor(out=yfin[:], in0=out_ps[:], in1=xfb_sb[:],
                                    op=OP.add)
            nc.sync.dma_start(out=yout[:], in_=yfin[:])

    return nc


# ------------------------------------------------------------------ driver

def _prepare(x, cond, edge_dist, edge_index, t, n_cores):
    B, N, _ = x.shape
    BN = B * N
    xf = np.ascontiguousarray(x.reshape(BN, 3).astype(np.float32))
    h = np.concatenate(
        [cond.reshape(BN, -1).astype(np.float32),
         np.full((BN, 1), float(t), np.float32)], axis=1)      # [BN, 64]

    src = np.asarray(edge_index[0], np.int64)
    dst = np.asarray(edge_index[1], np.int64)
    metas, nchunk, nblk, n_core, TL, hi_base, CPL, CPH = _plan(
        src, dst, np.asarray(edge_dist, np.float32), BN, n_cores)

    in_maps = []
    for m in metas:
        base = m["base"]
        xf_pad = np.zeros((nblk * BLK, 3), np.float32)
        xf_pad[:n_core] = xf[base:base + n_core]
        xfbl = np.ascontiguousarray(
            xf_pad.reshape(nblk, BLK, 3).transpose(0, 2, 1).reshape(nblk, 192))
        xf_hi = xf_pad.astype(ml_dtypes.bfloat16)
        r1 = xf_pad - np.asarray(xf_hi, np.float32)
        xf_mid = r1.astype(ml_dtypes.bfloat16)
        xf_lo = (r1 - np.asarray(xf_mid, np.float32)).astype(ml_dtypes.bfloat16)
        xfbt9 = np.concatenate(
            [np.asarray(xf_hi).reshape(nblk, BLK, 3),
             np.asarray(xf_mid).reshape(nblk, BLK, 3),
             np.asarray(xf_lo).reshape(nblk, BLK, 3)], axis=2)   # [nblk, BLK, 9]
        xfbt = np.ascontiguousarray(
            xfbt9.transpose(1, 0, 2).reshape(BLK, nblk * 9)).astype(ml_dtypes.bfloat16)
        in_maps.append(dict(
            idx16=m["idx16"], dl=m["dl"], dlrow=m["dlrow_bf"], drow=m["drow"],
            blockid=m["blockid"], xfb=xfbl, xfbt=xfbt,
        ))
    return in_maps, h, xf, nchunk, nblk, n_core, TL, hi_base, CPL, CPH, BN, (B, N)


def _fill_weights(in_maps, h, xf, nblk, n_core, BN,
                  ew1, eb1, ew2, eb2, cw1, cb1, cw2):
    cw1 = np.asarray(cw1, np.float32)
    w2c = (np.asarray(ew2, np.float32) @ cw1[128:160, :])
    cb1p = (np.asarray(cb1, np.float32)
            + np.asarray(eb2, np.float32) @ cw1[128:160, :])

    # gather table rows: [A = h @ cw1[0:64] bf16 x128 | xf f32 x3 | pad]
    A = (h @ cw1[0:64, :]).astype(ml_dtypes.bfloat16)           # [BN, 128]
    tb = np.zeros((BN, ROWE * 2), np.uint8)
    tb[:, 0:256] = np.ascontiguousarray(A).view(np.uint8)
    tb[:, 256:268] = xf.view(np.uint8)
    table = np.ascontiguousarray(tb).view(ml_dtypes.bfloat16)   # [BN, ROWE]

    shared = dict(
        table=table,
        w2c=np.ascontiguousarray(w2c).astype(ml_dtypes.bfloat16),
        ew1c=np.ascontiguousarray(
            np.asarray(ew1, np.float32).reshape(1, 32)).astype(ml_dtypes.bfloat16),
        cw2c=np.ascontiguousarray(
            np.asarray(cw2, np.float32).reshape(128, 1)).astype(ml_dtypes.bfloat16),
        eb1c=np.ascontiguousarray(np.asarray(eb1, np.float32).reshape(32, 1)),
        cb1p=np.ascontiguousarray(cb1p.reshape(128, 1)),
    )
    cw1d = cw1[64:128, :]
    for ci, m in enumerate(in_maps):
        base = ci * n_core
        h_pad = np.zeros((nblk * BLK, 64), np.float32)
        h_pad[:n_core] = h[base:base + n_core]
        btab = h_pad @ cw1d
        m["btab"] = np.ascontiguousarray(
            btab.reshape(nblk, BLK, 128).transpose(1, 0, 2).reshape(BLK, nblk * 128)
        ).astype(ml_dtypes.bfloat16)
        m.update(shared)


def _assemble(results, nblk, n_core, B, N):
    outs = []
    for r in results:
        y = r["yout"].reshape(nblk, 3, BLK).transpose(1, 0, 2).reshape(3, nblk * BLK)
        outs.append(y[:, :n_core])
    full = np.concatenate(outs, axis=1)          # [3, BN]
    return np.ascontiguousarray(full.T).reshape(B, N, 3)


def kernel(x, cond, edge_dist, ew1, eb1, ew2, eb2, nw1, nb1, nw2, nb2,
           cw1, cb1, cw2, edge_index, t, **_unused):
    x = np.asarray(x)
    cond = np.asarray(cond)
    (in_maps, h, xf, nchunk, nblk, n_core, TL, hi_base, CPL, CPH, BN,
     (B, N)) = _prepare(x, cond, np.asarray(edge_dist),
                        np.asarray(edge_index), t, N_CORES)
    _fill_weights(in_maps, h, xf, nblk, n_core, BN,
                  np.asarray(ew1), np.asarray(eb1), np.asarray(ew2),
                  np.asarray(eb2), np.asarray(cw1), np.asarray(cb1),
                  np.asarray(cw2))

    nc = build_bass(BN, nchunk, nblk, TL, hi_base, CPL, CPH, N_CORES)
    _split_ctrl_waits(nc)
    lower_extended_insts(nc)

    from concourse.bass_utils import run_bass_kernel_spmd
    res = run_bass_kernel_spmd(nc, in_maps, core_ids=list(range(N_CORES)),
                               trace=bool(int(os.environ.get("GNN_TRACE", "0"))))
    global LAST_RESULTS
    LAST_RESULTS = res
    out = _assemble(res.results, nblk, n_core, B, N)
    return out.astype(np.float32)


LAST_RESULTS = None


# revision 34
# speedup vs baseline: 1.4864x; 1.4864x over previous
"""Trainium2 Bass kernel for nn_EquivariantDiffuserV46 (GNN message passing).

Computation (node-MLP branch of the reference is dead code — output depends
only on the coord path):
    h = concat(cond, t)                    [BN, 64]
    edge_attr = silu(d*ew1+eb1) @ ew2+eb2  [E, 32]
    m = [h[src], h[dst], edge_attr]        [E, 160]
    cw = silu(m @ cw1 + cb1) @ cw2         [E, 1]
    upd = cw * (x[src]-x[dst]) / max(||x[src]-x[dst]||, 1e-8)
    out = x + segment_sum(upd, dst)

Strategy (v3): edges sorted by dst, dst-range sharded over 8 cores. Each
core fetches per-src rows [A=h@cw1[:64] bf16(128) | x f32(3)] (512B) with
batched dma_gather instructions striped over 4 SWDGE queues — one gather
per 2048-edge tile. Gathering the projection A instead of h lets the
src contribution enter the z PSUM via a single lhsT=A/rhs=identity
matmul (no transposes, no PSUM->SBUF copies). The dst side needs no
gather: edges are laid out with a compile-time-fixed number of chunks
per 64-node dst block, so h[dst]-dependent terms come from per-block
tables (Btab = Hblk @ cw1[64:128] f32, xfbT f32) via one-hot matmuls.
1/len uses the DVE bit-trick rsqrt (+2 Newton steps) so the ACT engine
only ever runs Silu (one table set, no table thrash). dma_gather
indices are int16, so chunks are split into two src classes (gather
base 0 vs BN-32768); class regions are tile-aligned.
"""
import os
import sys

for _p in ("/opt/trn_rl_repo",):
    if _p not in sys.path:
        sys.path.insert(0, _p)

import numpy as np
import ml_dtypes

from concourse import bass, mybir
from concourse.tile import TileContext
from concourse.masks import make_identity
from concourse import library_config
from concourse.library_overlay import lower_extended_insts

F32 = mybir.dt.float32
BF16 = mybir.dt.bfloat16
I32 = mybir.dt.int32
I16 = mybir.dt.int16
P = 128          # partitions / edges per chunk
BLK = 64         # nodes per dst block
CHT = 16         # chunks per tile (2048 edges per gather)
ROWE = 256       # gather row elements (bf16): 512B
N_CORES = 8
NQ = 4           # SWDGE queues for gathers
RSQRT_MAGIC = 0x5F3759DF


# ---------------------------------------------------------------- host prep

def _plan(src, dst, edge_dist, BN, n_cores):
    """Sort edges by dst, shard by dst range, lay out a uniform chunk stream.

    Chunk stream layout (identical shape for all cores; data differs):
      [ nblk blocks x CPL low-class chunks | tile pad | nblk x CPH high | pad ]
    Chunk c < TLC belongs to block c // CPL; chunk c >= TLC to block
    (c - TLC) // CPH (clamped). Every chunk holds 128 edges of one dst
    block and one src class; pad edges have dstloc=99 (dead one-hot).
    """
    n_core = BN // n_cores
    nblk = (n_core + BLK - 1) // BLK
    hi_base = BN - 32768            # high-class gather base
    lo_cut = hi_base                # src < lo_cut  -> rigid low
    hi_cut = 32768                  # src >= hi_cut -> rigid high

    order = np.argsort(dst, kind="stable")
    src_s = src[order]
    dst_s = dst[order]
    dist_s = edge_dist[order]
    bounds = np.searchsorted(dst_s, np.arange(0, BN + 1, n_core))

    percore = []
    need_l = need_h = need_t = 0
    for c in range(n_cores):
        lo, hi = bounds[c], bounds[c + 1]
        base = c * n_core
        cs, cd, cw = src_s[lo:hi], dst_s[lo:hi], dist_s[lo:hi]
        blk = (cd - base) // BLK
        rows = []
        for b in range(nblk):
            m = blk == b
            bs, bd, bw = cs[m], cd[m], cw[m]
            cls_rl = bs < lo_cut
            cls_rh = bs >= hi_cut
            cls_fx = ~cls_rl & ~cls_rh
            rows.append((bs, bd, bw, cls_rl, cls_rh, cls_fx))
            nl, nh, nf = int(cls_rl.sum()), int(cls_rh.sum()), int(cls_fx.sum())
            need_l = max(need_l, nl)
            need_h = max(need_h, nh)
            need_t = max(need_t, nl + nh + nf)
        percore.append((base, rows))

    CPL = (need_l + P - 1) // P
    CPH = (need_h + P - 1) // P
    while CPL * P + CPH * P < need_t:
        if CPL <= CPH:
            CPL += 1
        else:
            CPH += 1

    ncl = nblk * CPL
    nch = nblk * CPH
    TL = (ncl + CHT - 1) // CHT
    TH = (nch + CHT - 1) // CHT
    NT = TL + TH
    if NT % 8:                            # nslot = nchunk/128 integral
        TH += 8 - NT % 8
        NT = TL + TH
    nchunk = NT * CHT
    TLC = TL * CHT

    def blk_of_chunk(c):
        if c < TLC:
            return min(c // CPL, nblk - 1)
        return min((c - TLC) // CPH, nblk - 1)

    metas = []
    for base, rows in percore:
        idx = np.zeros(nchunk * P, np.int64)
        dloc = np.full(nchunk * P, 99.0, np.float32)
        dw = np.ones(nchunk * P, np.float32)
        for b in range(nblk):
            bs, bd, bw, rl, rh, fx = rows[b]
            nl_r = int(rl.sum())
            x_lo = min(int(fx.sum()), CPL * P - nl_r)
            fx_idx = np.nonzero(fx)[0]
            lo_m = rl.copy()
            lo_m[fx_idx[:x_lo]] = True
            hi_m = ~lo_m
            for cls, m, cp, coff, cbase in (
                (0, lo_m, CPL, b * CPL * P, 0),
                (1, hi_m, CPH, TLC * P + b * CPH * P, hi_base),
            ):
                es, ed, ew = bs[m], bd[m], bw[m]
                n = es.size
                assert n <= cp * P, (b, cls, n, cp * P)
                sl = slice(coff, coff + n)
                idx[sl] = es - cbase
                dloc[sl] = (ed - (base + b * BLK)).astype(np.float32)
                dw[sl] = ew

        idx16 = idx.astype(np.int16)
        assert (idx < 32768).all() and (idx >= 0).all()
        iw = np.zeros((P, NT * P), np.int16)
        for t in range(NT):
            blk16 = idx16[t * CHT * P:(t + 1) * CHT * P].reshape(CHT * P // 16, 16).T
            for rep in range(8):
                iw[rep * 16:(rep + 1) * 16, t * P:(t + 1) * P] = blk16

        bid = np.array([blk_of_chunk(c) for c in range(nchunk)], np.float32)
        metas.append(dict(
            idx16=np.ascontiguousarray(iw),
            dl=np.ascontiguousarray(dloc.reshape(nchunk, P).T),
            dlrow=np.ascontiguousarray(dloc.reshape(1, -1)),
            drow=np.ascontiguousarray(dw.reshape(1, -1).astype(ml_dtypes.bfloat16)),
            dlrow_bf=np.ascontiguousarray(
                dloc.reshape(1, -1).astype(ml_dtypes.bfloat16)),
            blockid=np.ascontiguousarray(bid.reshape(nchunk // P, P).T),
            base=base,
        ))
    return metas, nchunk, nblk, n_core, TL, hi_base, CPL, CPH


# ------------------------------------------------------------- bass builder

def _split_ctrl_waits(nc, limit=1):
    """Walrus in this toolchain rejects >limit sync waits on Drain-style ctrl
    instructions; move overflow waits onto preceding same-engine NoOps."""
    import bass_rust
    for fn in nc.m.functions:
        for bb in fn.blocks:
            out = []
            for inst in bb.instructions:
                si = inst.sync_info
                if (si is not None and si.on_wait
                        and len(si.on_wait) > limit):
                    waits = list(si.on_wait)
                    ups = list(si.on_update) if si.on_update else []
                    head, tail = waits[:-limit], waits[-limit:]
                    for k in range(0, len(head), limit):
                        nop = mybir.InstNoOp(name=f"{inst.name}-w{k}", ins=[], outs=[])
                        nop.engine = inst.engine
                        nop.sync_info = bass_rust.SyncInfo(
                            on_wait=head[k:k + limit], on_update=[])
                        out.append(nop)
                    inst.sync_info = bass_rust.SyncInfo(on_wait=tail, on_update=ups)
                out.append(inst)
            bb.instructions = out


def build_bass(BN, nchunk, nblk, TL, hi_base, CPL, CPH, n_cores=N_CORES):
    NT = nchunk // CHT
    nslot = nchunk // P
    epad = nchunk * P
    TLC = TL * CHT

    nc = bass.Bass("TRN2", target_bir_lowering=False, debug=False,
                   num_devices=n_cores, num_swdge_queues=NQ)

    table = nc.dram_tensor("table", [BN, ROWE], BF16, kind="ExternalInput")
    idx16 = nc.dram_tensor("idx16", [P, NT * P], I16, kind="ExternalInput")
    dl = nc.dram_tensor("dl", [P, nchunk], F32, kind="ExternalInput")
    dlrow = nc.dram_tensor("dlrow", [1, epad], BF16, kind="ExternalInput")
    drow = nc.dram_tensor("drow", [1, epad], BF16, kind="ExternalInput")
    blockid = nc.dram_tensor("blockid", [P, nslot], F32, kind="ExternalInput")
    xfb = nc.dram_tensor("xfb", [nblk, 192], F32, kind="ExternalInput")
    xfbt = nc.dram_tensor("xfbt", [BLK, nblk * 9], BF16, kind="ExternalInput")
    btab = nc.dram_tensor("btab", [BLK, nblk * 128], BF16, kind="ExternalInput")
    w2c = nc.dram_tensor("w2c", [32, 128], BF16, kind="ExternalInput")
    ew1c = nc.dram_tensor("ew1c", [1, 32], BF16, kind="ExternalInput")
    cw2c = nc.dram_tensor("cw2c", [128, 1], BF16, kind="ExternalInput")
    eb1c = nc.dram_tensor("eb1c", [32, 1], F32, kind="ExternalInput")
    cb1p = nc.dram_tensor("cb1p", [128, 1], F32, kind="ExternalInput")
    yout = nc.dram_tensor("yout", [nblk, 192], F32, kind="ExternalOutput")

    AF = mybir.ActivationFunctionType
    OP = mybir.AluOpType

    with TileContext(nc) as tc:
        with (
            tc.tile_pool(name="cst", bufs=1) as cst,
            tc.tile_pool(name="sb", bufs=2) as sbp,
            tc.tile_pool(name="gs", bufs=6) as gsp,
            tc.tile_pool(name="ps2", bufs=2, space="PSUM") as psp,
            tc.tile_pool(name="psxa", bufs=1, space="PSUM") as psxa,
            tc.tile_pool(name="ps1", bufs=1, space="PSUM") as psp1,
            tc.tile_pool(name="dr", bufs=1, space="DRAM") as drp,
        ):
            # ---------------- phase A: constants
            # (all gpsimd standard-library ops must precede load_library(mlp):
            # the reload replaces Q7 IRAM, and iota ucode lives in the
            # standard library)
            ident_bf = cst.tile([P, P], BF16)
            make_identity(nc, ident_bf)
            idx_sb = cst.tile([P, NT * P], I16)
            nc.sync.dma_start(out=idx_sb[:], in_=idx16[:])
            dl_sb = cst.tile([P, nchunk, 1], F32)
            nc.sync.dma_start(out=dl_sb[:, :, 0], in_=dl[:])
            blockid_sb = cst.tile([P, nslot], F32)
            nc.sync.dma_start(out=blockid_sb[:], in_=blockid[:])
            xfb_sb = cst.tile([nblk, 192], F32)
            nc.sync.dma_start(out=xfb_sb[:], in_=xfb[:])
            xfbt_sb = cst.tile([BLK, nblk * 9], BF16)
            nc.sync.dma_start(out=xfbt_sb[:], in_=xfbt[:])
            btab_sb = cst.tile([BLK, nblk * 128], BF16)
            nc.sync.dma_start(out=btab_sb[:], in_=btab[:])
            w2c_sb = cst.tile([32, 128], BF16)
            nc.sync.dma_start(out=w2c_sb[:], in_=w2c[:])
            ew1c_sb = cst.tile([1, 32], BF16)
            nc.sync.dma_start(out=ew1c_sb[:], in_=ew1c[:])
            cw2c_sb = cst.tile([128, 1], BF16)
            nc.sync.dma_start(out=cw2c_sb[:], in_=cw2c[:])
            eb1c_sb = cst.tile([32, 1], F32)
            nc.sync.dma_start(out=eb1c_sb[:], in_=eb1c[:])
            cb1p_sb = cst.tile([128, 1], F32)
            nc.sync.dma_start(out=cb1p_sb[:], in_=cb1p[:])

            iota64i = cst.tile([P, 1, BLK], I32)
            nc.gpsimd.iota(iota64i[:, 0, :], pattern=[[1, BLK]], base=0,
                           channel_multiplier=0)
            iota64 = cst.tile([P, 1, BLK], F32)
            nc.vector.tensor_copy(iota64[:], iota64i[:])
            iotaci = cst.tile([BLK, 1, 1], I32)
            nc.gpsimd.iota(iotaci[:, 0, :], pattern=[[1, 1]], base=0,
                           channel_multiplier=1)
            iotac = cst.tile([BLK, 1, 1], BF16)
            nc.vector.tensor_copy(iotac[:], iotaci[:])
            iotabi = cst.tile([P, nblk], I32)
            nc.gpsimd.iota(iotabi[:], pattern=[[1, nblk]], base=0,
                           channel_multiplier=0)
            iotab = cst.tile([P, nblk], F32)
            nc.vector.tensor_copy(iotab[:], iotabi[:])
            ones1 = cst.tile([1, BLK], BF16)
            nc.vector.memset(ones1[:], 1.0)
            magic = cst.tile([P, 1], I32)
            nc.vector.memset(magic[:], RSQRT_MAGIC)
            nidx_reg = nc.gpsimd.to_reg(CHT * P)   # shared by all gathers
            nc.gpsimd.load_library(library_config.mlp)

            ydram = drp.tile([nchunk, 192], F32)

            # ---------------- phase B: edge tiles
            NT_dbg = int(os.environ.get("GNN_MAXT", str(NT)))
            for t in range(min(NT, NT_dbg)):
                c0 = t * CHT
                in_ap = table[:] if t < TL else table[hi_base:, :]
                Gs = gsp.tile([P, CHT, ROWE], BF16, tag="Gs")
                nc.gpsimd.dma_gather(
                    out_ap=Gs[:], in_ap=in_ap,
                    idxs_ap=idx_sb[:, t * P:(t + 1) * P],
                    num_idxs=CHT * P, num_idxs_reg=nidx_reg, elem_size=ROWE,
                    transpose=False, single_packet=False, queue_num=t % NQ)
                dr_t = sbp.tile([1, CHT * P], BF16, tag="dr_t")
                nc.sync.dma_start(out=dr_t[:], in_=drow[:, c0 * P:(c0 + CHT) * P])
                dlr_t = sbp.tile([1, CHT * P], BF16, tag="dlr_t")
                nc.sync.dma_start(out=dlr_t[:], in_=dlrow[:, c0 * P:(c0 + CHT) * P])

                cwxd_ps = psp.tile([P, CHT, 4], F32, tag="cwxd")
                st_all = sbp.tile([BLK, CHT, P], BF16, tag="st")
                s_all = sbp.tile([P, CHT, BLK], BF16, tag="sall")
                for h2 in range(2):
                    dlrep_ps = psp1.tile([BLK, CHT * P // 2], F32, tag="dlrep")
                    for g2 in range(2):
                        o = (h2 * 2 + g2) * 512
                        nc.tensor.matmul(out=dlrep_ps[:, g2 * 512:(g2 + 1) * 512],
                                         lhsT=ones1[:],
                                         rhs=dlr_t[0:1, o:o + 512],
                                         start=True, stop=True)
                    nc.vector.tensor_tensor(
                        out=st_all[:, h2 * 8:(h2 + 1) * 8, :],
                        in0=dlrep_ps[:].rearrange("p (a b) -> p a b", a=CHT // 2),
                        in1=iotac[:].to_broadcast([BLK, CHT // 2, P]),
                        op=OP.is_equal)
                nc.vector.tensor_tensor(
                    out=s_all[:],
                    in0=iota64[:].to_broadcast([P, CHT, BLK]),
                    in1=dl_sb[:, c0:c0 + CHT, :].to_broadcast([P, CHT, BLK]),
                    op=OP.is_equal)

                for g in range(4):
                    u_ps = psp1.tile([32, 512], F32, tag="u")
                    nc.tensor.matmul(out=u_ps[:], lhsT=ew1c_sb[:],
                                     rhs=dr_t[0:1, g * 512:(g + 1) * 512],
                                     start=True, stop=True)
                    u_sb = sbp.tile([32, 512], BF16, tag="u_sb")
                    nc.scalar.activation(u_sb[:], u_ps[:], AF.Silu, bias=eb1c_sb[:])

                    z_ps = psp.tile([P, 512], F32, tag="z")
                    for c4 in range(4):
                        cc = g * 4 + c4
                        c = c0 + cc
                        if c < TLC:
                            b = min(c // CPL, nblk - 1)
                        else:
                            b = min((c - TLC) // CPH, nblk - 1)
                        # src contribution: z[:, e] += A[src_e]^T via identity
                        nc.tensor.matmul(
                            out=z_ps[:, c4 * P:(c4 + 1) * P],
                            lhsT=Gs[:, cc, 0:128], rhs=ident_bf[:],
                            start=(c4 == 0), stop=False)
                        st = st_all[:, cc, :]
                        nc.tensor.matmul(out=z_ps[:, c4 * P:(c4 + 1) * P],
                                         lhsT=btab_sb[:, b * 128:(b + 1) * 128],
                                         rhs=st, start=False, stop=False)
                        # xd = S_T @ (hi + mid + lo): accumulate in PSUM
                        nc.tensor.matmul(out=cwxd_ps[:, cc, 0:3],
                                         lhsT=st,
                                         rhs=xfbt_sb[:, b * 9:b * 9 + 3],
                                         start=True, stop=False)
                        nc.tensor.matmul(out=cwxd_ps[:, cc, 0:3],
                                         lhsT=st,
                                         rhs=xfbt_sb[:, b * 9 + 3:b * 9 + 6],
                                         start=False, stop=False)
                        nc.tensor.matmul(out=cwxd_ps[:, cc, 0:3],
                                         lhsT=st,
                                         rhs=xfbt_sb[:, b * 9 + 6:b * 9 + 9],
                                         start=False, stop=True)
                    nc.tensor.matmul(out=z_ps[:], lhsT=w2c_sb[:], rhs=u_sb[:],
                                     start=False, stop=True)
                    w_sb = sbp.tile([P, 512], BF16, tag="w_sb")
                    nc.scalar.activation(w_sb[:], z_ps[:], AF.Silu, bias=cb1p_sb[:])
                    for c4 in range(4):
                        cc = g * 4 + c4
                        nc.tensor.matmul(out=cwxd_ps[:, cc, 3:4],
                                         lhsT=w_sb[:, c4 * P:(c4 + 1) * P],
                                         rhs=cw2c_sb[:], start=True, stop=True)

                # coord update (row layout, all DVE — no ACT tables)
                dirt = sbp.tile([P, CHT, 3], F32, tag="dirt")
                nc.vector.tensor_tensor(
                    out=dirt[:], in0=Gs[:, :, 128:134].bitcast(F32),
                    in1=cwxd_ps[:, :, 0:3], op=OP.subtract)
                sq = sbp.tile([P, CHT, 3], F32, tag="sq")
                nc.vector.tensor_tensor(out=sq[:], in0=dirt[:], in1=dirt[:],
                                        op=OP.mult)
                ss = sbp.tile([P, CHT, 1], F32, tag="ss")
                nc.vector.tensor_reduce(out=ss[:, :, 0], in_=sq[:],
                                        axis=mybir.AxisListType.X, op=OP.add)
                # 1/sqrt(ss): clamp+shift fused in int domain (positive f32
                # ordering == int ordering), bit-trick seed + 2 Newton iters
                yv = sbp.tile([P, CHT, 1], F32, tag="yv")
                nc.vector.tensor_scalar_max(ss[:, :, 0], ss[:, :, 0], 1e-16)
                nc.vector.tensor_scalar(
                    out=yv[:, :, 0].bitcast(I32), in0=ss[:, :, 0].bitcast(I32),
                    scalar1=1, scalar2=None, op0=OP.logical_shift_right)
                nc.vector.tensor_tensor(
                    out=yv[:, :, 0].bitcast(I32),
                    in0=magic[:].to_broadcast([P, CHT]),
                    in1=yv[:, :, 0].bitcast(I32), op=OP.subtract)
                tv = sbp.tile([P, CHT, 1], F32, tag="tv")
                for _ in range(2):
                    nc.vector.tensor_tensor(out=tv[:], in0=yv[:], in1=yv[:],
                                            op=OP.mult)
                    nc.vector.tensor_tensor(out=tv[:], in0=tv[:], in1=ss[:],
                                            op=OP.mult)
                    nc.vector.tensor_scalar(
                        out=tv[:, :, 0], in0=tv[:, :, 0], scalar1=-0.5,
                        scalar2=1.5, op0=OP.mult, op1=OP.add)
                    nc.vector.tensor_tensor(out=yv[:], in0=yv[:], in1=tv[:],
                                            op=OP.mult)
                fac = sbp.tile([P, CHT, 1], F32, tag="fac")
                nc.vector.tensor_tensor(out=fac[:, :, 0], in0=yv[:, :, 0],
                                        in1=cwxd_ps[:, :, 3], op=OP.mult)
                upd_bf = sbp.tile([P, CHT, 3], BF16, tag="upd_bf")
                nc.vector.tensor_tensor(out=upd_bf[:], in0=dirt[:],
                                        in1=fac[:].to_broadcast([P, CHT, 3]),
                                        op=OP.mult)

                # one-hot scatter -> per-chunk [3, 64] node sums
                ystrip = sbp.tile([3, CHT, BLK], F32, tag="ystrip")
                for h in range(2):
                    xa_ps = psxa.tile([3, 8 * BLK], F32, tag="xa")
                    for c8 in range(8):
                        cc = h * 8 + c8
                        nc.tensor.matmul(out=xa_ps[:, c8 * BLK:(c8 + 1) * BLK],
                                         lhsT=upd_bf[:, cc, :],
                                         rhs=s_all[:, cc, :],
                                         start=True, stop=True)
                    nc.scalar.copy(ystrip[:, h * 8:(h + 1) * 8, :], xa_ps[:])
                nc.sync.dma_start(
                    out=ydram[c0:c0 + CHT, :].rearrange("q (k j) -> k q j", k=3),
                    in_=ystrip[:])

            # ---------------- phase C: block-stage reduction + x residual
            ysb = cst.tile([P, nslot, 192], F32)
            nc.sync.dma_start(out=ysb[:],
                              in_=ydram[:].rearrange("(s p) f -> p s f", p=P))
            out_ps = psp.tile([nblk, 192], F32, tag="z")
            for s in range(nslot):
                O = sbp.tile([P, nblk], F32, tag="O")
                nc.vector.tensor_tensor(
                    out=O[:], in0=iotab[:],
                    in1=blockid_sb[:, s:s + 1].to_broadcast([P, nblk]),
                    op=OP.is_equal)
                nc.tensor.matmul(out=out_ps[:], lhsT=O[:], rhs=ysb[:, s, :],
                                 start=(s == 0), stop=(s == nslot - 1))
            yfin = cst.tile([nblk, 192], F32)
            nc.vector.tensor_tensor(out=yfin[:], in0=out_ps[:], in1=xfb_sb[:],
                                    op=OP.add)
            nc.sync.dma_start(out=yout[:], in_=yfin[:])

    return nc


# ------------------------------------------------------------------ driver

def _prepare(x, cond, edge_dist, edge_index, t, n_cores):
    B, N, _ = x.shape
    BN = B * N
    xf = np.ascontiguousarray(x.reshape(BN, 3).astype(np.float32))
    h = np.concatenate(
        [cond.reshape(BN, -1).astype(np.float32),
         np.full((BN, 1), float(t), np.float32)], axis=1)      # [BN, 64]

    src = np.asarray(edge_index[0], np.int64)
    dst = np.asarray(edge_index[1], np.int64)
    metas, nchunk, nblk, n_core, TL, hi_base, CPL, CPH = _plan(
        src, dst, np.asarray(edge_dist, np.float32), BN, n_cores)

    in_maps = []
    for m in metas:
        base = m["base"]
        xf_pad = np.zeros((nblk * BLK, 3), np.float32)
        xf_pad[:n_core] = xf[base:base + n_core]
        xfbl = np.ascontiguousarray(
            xf_pad.reshape(nblk, BLK, 3).transpose(0, 2, 1).reshape(nblk, 192))
        xf_hi = xf_pad.astype(ml_dtypes.bfloat16)
        r1 = xf_pad - np.asarray(xf_hi, np.float32)
        xf_mid = r1.astype(ml_dtypes.bfloat16)
        xf_lo = (r1 - np.asarray(xf_mid, np.float32)).astype(ml_dtypes.bfloat16)
        xfbt9 = np.concatenate(
            [np.asarray(xf_hi).reshape(nblk, BLK, 3),
             np.asarray(xf_mid).reshape(nblk, BLK, 3),
             np.asarray(xf_lo).reshape(nblk, BLK, 3)], axis=2)   # [nblk, BLK, 9]
        xfbt = np.ascontiguousarray(
            xfbt9.transpose(1, 0, 2).reshape(BLK, nblk * 9)).astype(ml_dtypes.bfloat16)
        in_maps.append(dict(
            idx16=m["idx16"], dl=m["dl"], dlrow=m["dlrow_bf"], drow=m["drow"],
            blockid=m["blockid"], xfb=xfbl, xfbt=xfbt,
        ))
    return in_maps, h, xf, nchunk, nblk, n_core, TL, hi_base, CPL, CPH, BN, (B, N)


def _fill_weights(in_maps, h, xf, nblk, n_core, BN,
                  ew1, eb1, ew2, eb2, cw1, cb1, cw2):
    cw1 = np.asarray(cw1, np.float32)
    w2c = (np.asarray(ew2, np.float32) @ cw1[128:160, :])
    cb1p = (np.asarray(cb1, np.float32)
            + np.asarray(eb2, np.float32) @ cw1[128:160, :])

    # gather table rows: [A = h @ cw1[0:64] bf16 x128 | xf f32 x3 | pad]
    A = (h @ cw1[0:64, :]).astype(ml_dtypes.bfloat16)           # [BN, 128]
    tb = np.zeros((BN, ROWE * 2), np.uint8)
    tb[:, 0:256] = np.ascontiguousarray(A).view(np.uint8)
    tb[:, 256:268] = xf.view(np.uint8)
    table = np.ascontiguousarray(tb).view(ml_dtypes.bfloat16)   # [BN, ROWE]

    shared = dict(
        table=table,
        w2c=np.ascontiguousarray(w2c).astype(ml_dtypes.bfloat16),
        ew1c=np.ascontiguousarray(
            np.asarray(ew1, np.float32).reshape(1, 32)).astype(ml_dtypes.bfloat16),
        cw2c=np.ascontiguousarray(
            np.asarray(cw2, np.float32).reshape(128, 1)).astype(ml_dtypes.bfloat16),
        eb1c=np.ascontiguousarray(np.asarray(eb1, np.float32).reshape(32, 1)),
        cb1p=np.ascontiguousarray(cb1p.reshape(128, 1)),
    )
    cw1d = cw1[64:128, :]
    for ci, m in enumerate(in_maps):
        base = ci * n_core
        h_pad = np.zeros((nblk * BLK, 64), np.float32)
        h_pad[:n_core] = h[base:base + n_core]
        btab = h_pad @ cw1d
        m["btab"] = np.ascontiguousarray(
            btab.reshape(nblk, BLK, 128).transpose(1, 0, 2).reshape(BLK, nblk * 128)
        ).astype(ml_dtypes.bfloat16)
        m.update(shared)


def _assemble(results, nblk, n_core, B, N):
    outs = []
    for r in results:
        y = r["yout"].reshape(nblk, 3, BLK).transpose(1, 0, 2).reshape(3, nblk * BLK)
        outs.append(y[:, :n_core])
    full = np.concatenate(outs, axis=1)          # [3, BN]
    return np.ascontiguousarray(full.T).reshape(B, N, 3)


def kernel(x, cond, edge_dist, ew1, eb1, ew2, eb2, nw1, nb1, nw2, nb2,
           cw1, cb1, cw2, edge_index, t, **_unused):
    x = np.asarray(x)
    cond = np.asarray(cond)
    (in_maps, h, xf, nchunk, nblk, n_core, TL, hi_base, CPL, CPH, BN,
     (B, N)) = _prepare(x, cond, np.asarray(edge_dist),
                        np.asarray(edge_index), t, N_CORES)
    _fill_weights(in_maps, h, xf, nblk, n_core, BN,
                  np.asarray(ew1), np.asarray(eb1), np.asarray(ew2),
                  np.asarray(eb2), np.asarray(cw1), np.asarray(cb1),
                  np.asarray(cw2))

    nc = build_bass(BN, nchunk, nblk, TL, hi_base, CPL, CPH, N_CORES)
    _split_ctrl_waits(nc)
    lower_extended_insts(nc)

    from concourse.bass_utils import run_bass_kernel_spmd
    res = run_bass_kernel_spmd(nc, in_maps, core_ids=list(range(N_CORES)),
                               trace=bool(int(os.environ.get("GNN_TRACE", "0"))))
    global LAST_RESULTS
    LAST_RESULTS = res
    out = _assemble(res.results, nblk, n_core, B, N)
    return out.astype(np.float32)


LAST_RESULTS = None
